# revision 1
# baseline (speedup 1.0000x reference)
"""Trainium2 Bass kernel: 3-layer GraphConv GNN encoder (mean aggregation).

reference math (PyG GraphConv, aggr='mean'):
    h1 = relu(mean_agg(x) @ w1_rel + b1 + x @ w1_root)
    h2 = relu(mean_agg(h1) @ w2_rel + b2 + h1 @ w2_root)
    mu = mean_agg(h2) @ wmu_rel + bmu + h2 @ wmu_root
    ls = mean_agg(h2) @ wls_rel + bls + h2 @ wls_root

Mean aggregation is linear, so it commutes with the dense projections.
We aggregate in the *smallest* feature dim per layer:
    L1: aggregate x (128 wide), then project.
    L2: q2 = h1 @ w2_rel (512 wide): mean_agg(h1)@w2_rel == mean_agg(q2)
    L3: q3 = h2 @ [wmu_rel|wls_rel] (16 wide), aggregate q3.

Distribution: nodes sharded as contiguous ranges of 2500 over 8 cores. Edges
partitioned by destination core; per-core edges grouped by 128-node
destination tile and padded to 128-edge blocks (host, index-only
preprocessing). Gather of source features via gpsimd dma_gather from an HBM
table; segment-sum via one-hot matmuls (one-hot built on DVE from
destination ids); mean scale (1/deg) applied to the aggregated [feat, nodes]
tile columns after accumulation. q2/q3 tables are AllGathered across cores
between layers.
"""

import numpy as np

import concourse.bass as bass
import concourse.mybir as mybir
import concourse.tile as tile
from concourse import bacc
from concourse.bass_utils import run_bass_kernel_spmd
from concourse.masks import make_identity

P = 128
FP32 = mybir.dt.float32
BF16 = mybir.dt.bfloat16
I16 = mybir.dt.int16
AF = mybir.ActivationFunctionType
ALU = mybir.AluOpType


class Cfg:
    def __init__(self, n_nodes=20000, n_edges=160000, f_in=128, h1=1024, h2=512,
                 out=8, n_cores=8):
        assert n_nodes % n_cores == 0
        self.n = n_nodes
        self.e = n_edges
        self.f = f_in
        self.h1 = h1
        self.h2 = h2
        self.out = out
        self.nc = n_cores
        self.own = n_nodes // n_cores              # real nodes per core
        self.nt = (self.own + P - 1) // P          # dst tiles per core
        self.own_pad = self.nt * P                 # padded nodes per core
        self.gsz = min(512, self.own_pad)          # node-group width for dense matmuls
        assert self.own_pad % self.gsz == 0
        self.ng = self.own_pad // self.gsz
        self.tpg = self.gsz // P                   # tiles per group
        self.h1c = h1 // P                         # H1 chunks of 128
        self.h2c = h2 // P                         # H2 chunks of 128
        self.oc = 2 * out                          # mu|logstd concat width (16)


def _wrap_idx(a, dtype=np.int16):
    """dma_gather index layout: idx j at [j%16, j//16], replicated to 128 partitions."""
    nb16 = a.shape[0] // 16
    w = a.reshape(nb16, 16).T.astype(dtype)        # [16, nb16]
    return np.tile(w, (8, 1))                      # [128, nb16]


def shard_graph(cfg: Cfg, edge_index):
    """Partition/pad edges by (dst core, dst tile). Returns per-core index
    arrays (equal shapes across cores) + shared per-tile block counts NB."""
    src = np.asarray(edge_index[0], dtype=np.int64)
    dst = np.asarray(edge_index[1], dtype=np.int64)
    order = np.argsort(dst, kind="stable")
    src_s = src[order]
    dst_s = dst[order]

    bounds = []
    for c in range(cfg.nc):
        for t in range(cfg.nt):
            bounds.append(c * cfg.own + t * P)
    bounds.append(cfg.n)
    seg = np.searchsorted(dst_s, np.asarray(bounds))
    cnt = np.diff(seg).reshape(cfg.nc, cfg.nt)

    NB = np.maximum(1, (cnt.max(axis=0) + P - 1) // P).astype(int)   # per tile t
    nbtot = int(NB.sum())

    per_core = []
    for c in range(cfg.nc):
        srcpad = np.zeros(nbtot * P, dtype=np.int64)
        dstloc = np.full(nbtot * P, -1.0, dtype=np.float32)
        off = 0
        for t in range(cfg.nt):
            k = c * cfg.nt + t
            s0, s1 = seg[k], seg[k + 1]
            m = s1 - s0
            srcpad[off:off + m] = src_s[s0:s1]
            dstloc[off:off + m] = (dst_s[s0:s1] - (c * cfg.own + t * P)).astype(np.float32)
            off += NB[t] * P
        src_remap = (srcpad // cfg.own) * cfg.own_pad + (srcpad % cfg.own)
        per_core.append({
            "idx1": _wrap_idx(srcpad),                        # [128, nbtot*8] int16
            "idx23": _wrap_idx(src_remap),                    # [128, nbtot*8] int16
            "dstloc": dstloc.reshape(nbtot, P).T.copy(),      # [128, nbtot] f32
        })
    return per_core, NB


def host_prep(cfg: Cfg, inputs):
    """Build per-core in_maps. Pure layout work (slice/pad/transpose/concat)."""
    x = np.asarray(inputs["x"], dtype=np.float32)
    per_core_idx, NB = shard_graph(cfg, inputs["edge_index"])

    w3rel = np.concatenate([np.asarray(inputs["wmu_rel"]), np.asarray(inputs["wls_rel"])], axis=1).astype(np.float32)
    w3root = np.concatenate([np.asarray(inputs["wmu_root"]), np.asarray(inputs["wls_root"])], axis=1).astype(np.float32)
    b3 = np.concatenate([np.asarray(inputs["bmu"]), np.asarray(inputs["bls"])]).astype(np.float32)
    b3T = np.zeros((P, 1), dtype=np.float32)
    b3T[:cfg.oc, 0] = b3
    b1T = np.asarray(inputs["b1"], dtype=np.float32).reshape(cfg.h1c, P).T.copy()
    b2T = np.asarray(inputs["b2"], dtype=np.float32).reshape(cfg.h2c, P).T.copy()

    in_maps = []
    for c in range(cfg.nc):
        xo = x[c * cfg.own:(c + 1) * cfg.own]
        xT = np.zeros((cfg.f, cfg.own_pad), dtype=np.float32)
        xT[:, :cfg.own] = xo.T
        m = dict(per_core_idx[c])
        m.update({
            "xg": x,
            "xT": xT,
            "w1rel": np.asarray(inputs["w1_rel"], dtype=np.float32),
            "w1root": np.asarray(inputs["w1_root"], dtype=np.float32),
            "w2rel": np.asarray(inputs["w2_rel"], dtype=np.float32),
            "w2root": np.asarray(inputs["w2_root"], dtype=np.float32),
            "w3rel": w3rel,
            "w3root": w3root,
            "b1T": b1T,
            "b2T": b2T,
            "b3T": b3T,
        })
        in_maps.append(m)
    return in_maps, NB


class _StageCutExc(Exception):
    pass


_StageCut = _StageCutExc()


def build_kernel(cfg: Cfg, NB, stage=99):
    """Emit the Bass program (same for all cores)."""
    nbtot = int(sum(NB))
    nbmax = int(max(NB))
    nc = bacc.Bacc("TRN2", target_bir_lowering=False, debug=False,
                   num_devices=cfg.nc)

    # ---- I/O ----
    d_xg = nc.dram_tensor("xg", [cfg.n, cfg.f], FP32, kind="ExternalInput")
    d_xT = nc.dram_tensor("xT", [cfg.f, cfg.own_pad], FP32, kind="ExternalInput")
    d_idx1 = nc.dram_tensor("idx1", [P, nbtot * 8], I16, kind="ExternalInput")
    d_idx23 = nc.dram_tensor("idx23", [P, nbtot * 8], I16, kind="ExternalInput")
    d_dstloc = nc.dram_tensor("dstloc", [P, nbtot], FP32, kind="ExternalInput")
    d_w1rel = nc.dram_tensor("w1rel", [cfg.f, cfg.h1], FP32, kind="ExternalInput")
    d_w1root = nc.dram_tensor("w1root", [cfg.f, cfg.h1], FP32, kind="ExternalInput")
    d_w2rel = nc.dram_tensor("w2rel", [cfg.h1, cfg.h2], FP32, kind="ExternalInput")
    d_w2root = nc.dram_tensor("w2root", [cfg.h1, cfg.h2], FP32, kind="ExternalInput")
    d_w3rel = nc.dram_tensor("w3rel", [cfg.h2, cfg.oc], FP32, kind="ExternalInput")
    d_w3root = nc.dram_tensor("w3root", [cfg.h2, cfg.oc], FP32, kind="ExternalInput")
    d_b1T = nc.dram_tensor("b1T", [P, cfg.h1c], FP32, kind="ExternalInput")
    d_b2T = nc.dram_tensor("b2T", [P, cfg.h2c], FP32, kind="ExternalInput")
    d_b3T = nc.dram_tensor("b3T", [P, 1], FP32, kind="ExternalInput")
    d_out = nc.dram_tensor("outT", [cfg.oc, cfg.own_pad], FP32, kind="ExternalOutput")

    rg = [list(range(cfg.nc))]

    with tile.TileContext(nc) as tc:
        with (
            tc.tile_pool(name="const", bufs=1) as cpool,
            tc.tile_pool(name="wts", bufs=1) as wpool,
            tc.tile_pool(name="resid", bufs=1) as rpool,
            tc.tile_pool(name="wtmp", bufs=2) as wtmp_pool,
            tc.tile_pool(name="gat", bufs=2) as gpool,
            tc.tile_pool(name="mwork", bufs=2) as mpool,
            tc.tile_pool(name="stage", bufs=2) as spool,
            tc.tile_pool(name="psA", bufs=2, space="PSUM") as psA,
            tc.tile_pool(name="psB", bufs=2, space="PSUM") as psB,
            tc.tile_pool(name="psC", bufs=2, space="PSUM") as psC,
            tc.tile_pool(name="dram", bufs=1, space="DRAM") as dpool,
        ):
            try:
                # ---- constants ----
                iotaB = cpool.tile([P, P], FP32)
                nc.gpsimd.iota(iotaB[:], pattern=[[1, P]], base=0, channel_multiplier=0,
                               allow_small_or_imprecise_dtypes=True)
                ones_e = cpool.tile([P, 1], FP32)
                nc.vector.memset(ones_e[:], 1.0)
                idb = cpool.tile([P, P], BF16)
                make_identity(nc, idb[:])

                if stage < 1:
                    z = cpool.tile([cfg.oc, 1], FP32, name="znull")
                    nc.sync.dma_start(out=z[:], in_=d_b3T[:cfg.oc, :])
                    nc.sync.dma_start(out=d_out[:cfg.oc, 0:1], in_=z[:])
                    raise _StageCut
                # ---- load + cast weights/biases ----
                def load_cast(dram_ap, rows, cols, dst_ap):
                    t = wtmp_pool.tile([P, cols], FP32, tag="wtmp")
                    nc.sync.dma_start(out=t[:rows, :], in_=dram_ap)
                    nc.vector.tensor_copy(out=dst_ap, in_=t[:rows, :])

                w1relb = wpool.tile([P, cfg.h1], BF16)
                w1rootb = wpool.tile([P, cfg.h1], BF16)
                load_cast(d_w1rel[:, :], cfg.f, cfg.h1, w1relb[:])
                load_cast(d_w1root[:, :], cfg.f, cfg.h1, w1rootb[:])

                w2relb = wpool.tile([P, cfg.h1c * cfg.h2], BF16)
                w2rootb = wpool.tile([P, cfg.h1c * cfg.h2], BF16)
                for k in range(cfg.h1c):
                    load_cast(d_w2rel[k * P:(k + 1) * P, :], P, cfg.h2,
                              w2relb[:, k * cfg.h2:(k + 1) * cfg.h2])
                    load_cast(d_w2root[k * P:(k + 1) * P, :], P, cfg.h2,
                              w2rootb[:, k * cfg.h2:(k + 1) * cfg.h2])

                w3relb = wpool.tile([P, cfg.h2c * cfg.oc], BF16)
                w3rootb = wpool.tile([P, cfg.h2c * cfg.oc], BF16)
                for k in range(cfg.h2c):
                    load_cast(d_w3rel[k * P:(k + 1) * P, :], P, cfg.oc,
                              w3relb[:, k * cfg.oc:(k + 1) * cfg.oc])
                    load_cast(d_w3root[k * P:(k + 1) * P, :], P, cfg.oc,
                              w3rootb[:, k * cfg.oc:(k + 1) * cfg.oc])

                b1T = cpool.tile([P, cfg.h1c], FP32)
                nc.sync.dma_start(out=b1T[:], in_=d_b1T[:, :])
                b2T = cpool.tile([P, cfg.h2c], FP32)
                nc.sync.dma_start(out=b2T[:], in_=d_b2T[:, :])
                b3T = cpool.tile([P, 1], FP32)
                nc.sync.dma_start(out=b3T[:], in_=d_b3T[:, :])

                # own-node features, feature-major, bf16 (chunked cast)
                xTb = rpool.tile([P, cfg.own_pad], BF16)
                for j in range(0, cfg.own_pad, 512):
                    w = min(512, cfg.own_pad - j)
                    load_cast(d_xT[:, j:j + w], P, w, xTb[:, j:j + w])

                # indices
                idx1 = rpool.tile([P, nbtot * 8], I16)
                nc.sync.dma_start(out=idx1[:], in_=d_idx1[:, :])
                idx23 = rpool.tile([P, nbtot * 8], I16)
                nc.sync.dma_start(out=idx23[:], in_=d_idx23[:, :])
                dstloc = rpool.tile([P, nbtot], FP32)
                nc.sync.dma_start(out=dstloc[:], in_=d_dstloc[:, :])

                # residents
                m1T = rpool.tile([P, cfg.own_pad], BF16)    # aggregated L1 (feature-major)
                h1T = rpool.tile([P, cfg.h1c * cfg.own_pad], BF16)
                h2T = rpool.tile([P, cfg.h2c * cfg.own_pad], BF16)
                invB_all = rpool.tile([P, cfg.nt * P], FP32)

                base = [int(sum(NB[:t])) for t in range(cfg.nt)]

                def keep(ap):
                    # anchor intermediate result to the output so DCE keeps the work
                    nc.gpsimd.dma_start(out=d_out[:cfg.oc, :P], in_=ap)

                # ================= Layer 1: deg + mean(x) ========================
                for t in range(cfg.nt):
                    nb = int(NB[t])
                    b0 = base[t]
                    G1 = gpool.tile([P, nbmax * cfg.h2 // 2], FP32, tag="G", name="G1")
                    for c0 in range(0, nb, 8):
                        cn = min(8, nb - c0)
                        nc.gpsimd.dma_gather(
                            out_ap=G1[:, c0 * cfg.f:(c0 + cn) * cfg.f]
                                .rearrange("p (b e) -> p b e", e=cfg.f),
                            in_ap=d_xg[:, :],
                            idxs_ap=idx1[:, (b0 + c0) * 8:(b0 + c0 + cn) * 8],
                            num_idxs=cn * P,
                            num_idxs_reg=cn * P,
                            elem_size=cfg.f,
                        )
                    Mf = mpool.tile([P, nbmax * P], FP32, tag="Mf", name="Mf")
                    degP = psC.tile([1, P], FP32, tag="small", name="degP")
                    for b in range(nb):
                        nc.vector.tensor_tensor(
                            out=Mf[:, b * P:(b + 1) * P],
                            in0=dstloc[:, b0 + b:b0 + b + 1].to_broadcast([P, P]),
                            in1=iotaB[:],
                            op=ALU.is_equal,
                        )
                        nc.tensor.matmul(degP[:], lhsT=ones_e[:], rhs=Mf[:, b * P:(b + 1) * P],
                                         start=(b == 0), stop=(b == nb - 1))
                    degS = spool.tile([1, P], FP32, tag="degS", name="degS")
                    nc.vector.tensor_scalar_max(out=degS[:], in0=degP[:], scalar1=1.0)
                    inv_row = spool.tile([1, P], FP32, tag="inv_row", name="inv_row")
                    nc.vector.reciprocal(out=inv_row[:], in_=degS[:])
                    invB = invB_all[:, t * P:(t + 1) * P]
                    nc.gpsimd.partition_broadcast(invB, inv_row[:], channels=P)

                    m1P = psB.tile([P, P], FP32, tag="t128", name="m1P")
                    for b in range(nb):
                        nc.tensor.matmul(
                            m1P[:],
                            lhsT=G1[:, b * cfg.f:(b + 1) * cfg.f],
                            rhs=Mf[:, b * P:(b + 1) * P],
                            start=(b == 0), stop=(b == nb - 1),
                        )
                    # mean = agg * (1/deg) per node column; cast to bf16
                    nc.vector.tensor_tensor(out=m1T[:, t * P:(t + 1) * P],
                                            in0=m1P[:], in1=invB, op=ALU.mult)

                if stage < 2:
                    keep(m1T[:cfg.oc, :P])
                    raise _StageCut
                # ---- L1 dense: h1 = relu(m1 @ w1rel + x @ w1root + b1) ----
                for g in range(cfg.ng):
                    gs = bass.ds(g * cfg.gsz, cfg.gsz)
                    for c in range(cfg.h1c):
                        h1P = psA.tile([P, cfg.gsz], FP32, tag="big", name="h1P")
                        nc.tensor.matmul(h1P[:], lhsT=w1relb[:, c * P:(c + 1) * P],
                                         rhs=m1T[:, gs], start=True, stop=False)
                        nc.tensor.matmul(h1P[:], lhsT=w1rootb[:, c * P:(c + 1) * P],
                                         rhs=xTb[:, gs], start=False, stop=True)
                        nc.scalar.activation(
                            out=h1T[:, c * cfg.own_pad + g * cfg.gsz:
                                    c * cfg.own_pad + (g + 1) * cfg.gsz],
                            in_=h1P[:], func=AF.Relu, bias=b1T[:, c:c + 1], scale=1.0)

                if stage < 3:
                    keep(h1T[:cfg.oc, :P])
                    raise _StageCut
                # ================= Layer 2 =================
                q2b = dpool.tile([cfg.own_pad * cfg.h2], BF16, name="q2bounce")
                q2t = dpool.tile([cfg.nc * cfg.own_pad * cfg.h2], BF16,
                                 name="q2tab", addr_space="Shared")
                q2b2d = q2b[:].rearrange("(r e) -> r e", e=cfg.h2)
                q2t2d = q2t[:].rearrange("(r e) -> r e", e=cfg.h2)

                for g in range(cfg.ng):
                    q2s = spool.tile([P, cfg.h2c * cfg.gsz], BF16, tag="q2s", name="q2s")
                    for c in range(cfg.h2c):
                        q2P = psA.tile([P, cfg.gsz], FP32, tag="big", name="q2P")
                        for k in range(cfg.h1c):
                            nc.tensor.matmul(
                                q2P[:],
                                lhsT=w2relb[:, k * cfg.h2 + c * P: k * cfg.h2 + (c + 1) * P],
                                rhs=h1T[:, k * cfg.own_pad + g * cfg.gsz:
                                        k * cfg.own_pad + (g + 1) * cfg.gsz],
                                start=(k == 0), stop=(k == cfg.h1c - 1))
                        nc.scalar.activation(out=q2s[:, c * cfg.gsz:(c + 1) * cfg.gsz],
                                             in_=q2P[:], func=AF.Copy)
                    # transpose [h2-chunk, node-subtile] -> node-major rows
                    q2n = spool.tile([P, cfg.tpg, cfg.h2], BF16, tag="q2n", name="q2n")
                    for s in range(cfg.tpg):
                        for c in range(cfg.h2c):
                            tp = psB.tile([P, P], BF16, tag="t128", name="tp")
                            nc.tensor.transpose(
                                tp[:], in_=q2s[:, c * cfg.gsz + s * P: c * cfg.gsz + (s + 1) * P],
                                identity=idb[:])
                            nc.vector.tensor_copy(out=q2n[:, s, c * P:(c + 1) * P], in_=tp[:])
                    nc.sync.dma_start(
                        out=q2b2d[g * cfg.gsz:(g + 1) * cfg.gsz, :]
                            .rearrange("(s p) e -> p s e", p=P),
                        in_=q2n[:])

                nc.gpsimd.collective_compute(
                    "AllGather", ALU.bypass, replica_groups=rg,
                    ins=[q2b[:]], outs=[q2t[:]])

                if stage < 4:
                    keep(m1T[:cfg.oc, :P])
                    raise _StageCut
                # scatter (mean of q2) + root + relu -> h2
                for t in range(cfg.nt):
                    nb = int(NB[t])
                    b0 = base[t]
                    invB = invB_all[:, t * P:(t + 1) * P]
                    G2 = gpool.tile([P, nbmax * cfg.h2], BF16, tag="G", name="G2")
                    for c0 in range(0, nb, 8):
                        cn = min(8, nb - c0)
                        nc.gpsimd.dma_gather(
                            out_ap=G2[:, c0 * cfg.h2:(c0 + cn) * cfg.h2]
                                .rearrange("p (b e) -> p b e", e=cfg.h2),
                            in_ap=q2t2d,
                            idxs_ap=idx23[:, (b0 + c0) * 8:(b0 + c0 + cn) * 8],
                            num_idxs=cn * P,
                            num_idxs_reg=cn * P,
                            elem_size=cfg.h2,
                        )
                    Mfb = mpool.tile([P, nbmax * P], BF16, tag="Mfb", name="Mfb")
                    for b in range(nb):
                        nc.vector.tensor_tensor(
                            out=Mfb[:, b * P:(b + 1) * P],
                            in0=dstloc[:, b0 + b:b0 + b + 1].to_broadcast([P, P]),
                            in1=iotaB[:],
                            op=ALU.is_equal,
                        )
                    m2P = psA.tile([P, cfg.h2], FP32, tag="big", name="m2P")
                    m2s = spool.tile([P, cfg.h2], BF16, tag="m2s", name="m2s")
                    for c in range(cfg.h2c):
                        cs = bass.ds(c * P, P)
                        for b in range(nb):
                            nc.tensor.matmul(
                                m2P[:, cs],
                                lhsT=G2[:, b * cfg.h2 + c * P: b * cfg.h2 + (c + 1) * P],
                                rhs=Mfb[:, b * P:(b + 1) * P],
                                start=(b == 0), stop=(b == nb - 1))
                        nc.vector.tensor_tensor(out=m2s[:, cs], in0=m2P[:, cs],
                                                in1=invB, op=ALU.mult)
                    r2P = psA.tile([P, cfg.h2], FP32, tag="big", name="r2P")
                    for c in range(cfg.h2c):
                        cs = bass.ds(c * P, P)
                        for k in range(cfg.h1c):
                            nc.tensor.matmul(
                                r2P[:, cs],
                                lhsT=w2rootb[:, k * cfg.h2 + c * P: k * cfg.h2 + (c + 1) * P],
                                rhs=h1T[:, k * cfg.own_pad + t * P:
                                        k * cfg.own_pad + (t + 1) * P],
                                start=(k == 0), stop=False)
                        nc.tensor.matmul(r2P[:, cs], lhsT=idb[:], rhs=m2s[:, cs],
                                         start=False, stop=True)
                        nc.scalar.activation(
                            out=h2T[:, c * cfg.own_pad + t * P: c * cfg.own_pad + (t + 1) * P],
                            in_=r2P[:, cs], func=AF.Relu, bias=b2T[:, c:c + 1], scale=1.0)

                if stage < 5:
                    keep(h2T[:cfg.oc, :P])
                    raise _StageCut
                # ================= Layer 3 =================
                q3b = dpool.tile([cfg.own_pad * P], BF16, name="q3bounce")
                q3t = dpool.tile([cfg.nc * cfg.own_pad * P], BF16,
                                 name="q3tab", addr_space="Shared")
                q3b2d = q3b[:].rearrange("(r e) -> r e", e=P)
                q3t2d = q3t[:].rearrange("(r e) -> r e", e=P)

                for t in range(cfg.nt):
                    q3P = psC.tile([cfg.oc, P], FP32, tag="small", name="q3P")
                    for k in range(cfg.h2c):
                        nc.tensor.matmul(
                            q3P[:],
                            lhsT=w3relb[:, k * cfg.oc:(k + 1) * cfg.oc],
                            rhs=h2T[:, k * cfg.own_pad + t * P: k * cfg.own_pad + (t + 1) * P],
                            start=(k == 0), stop=(k == cfg.h2c - 1))
                    q3s = spool.tile([cfg.oc, P], BF16, tag="q3s", name="q3s")
                    nc.vector.tensor_copy(out=q3s[:], in_=q3P[:])
                    tp3 = psB.tile([P, cfg.oc], BF16, tag="t128", name="tp3")
                    nc.tensor.transpose(tp3[:], in_=q3s[:], identity=idb[:cfg.oc, :cfg.oc])
                    q3n = spool.tile([P, P], BF16, tag="q3n", name="q3n")
                    nc.vector.memset(q3n[:], 0.0)
                    nc.vector.tensor_copy(out=q3n[:, :cfg.oc], in_=tp3[:])
                    nc.sync.dma_start(out=q3b2d[t * P:(t + 1) * P, :], in_=q3n[:])

                if stage < 6:
                    keep(h2T[:cfg.oc, :P])
                    raise _StageCut
                nc.gpsimd.collective_compute(
                    "AllGather", ALU.bypass, replica_groups=rg,
                    ins=[q3b[:]], outs=[q3t[:]])

                for t in range(cfg.nt):
                    nb = int(NB[t])
                    b0 = base[t]
                    invB = invB_all[:, t * P:(t + 1) * P]
                    G3 = gpool.tile([P, nbmax * P], BF16, tag="G", name="G3")
                    for c0 in range(0, nb, 8):
                        cn = min(8, nb - c0)
                        nc.gpsimd.dma_gather(
                            out_ap=G3[:, c0 * P:(c0 + cn) * P]
                                .rearrange("p (b e) -> p b e", e=P),
                            in_ap=q3t2d,
                            idxs_ap=idx23[:, (b0 + c0) * 8:(b0 + c0 + cn) * 8],
                            num_idxs=cn * P,
                            num_idxs_reg=cn * P,
                            elem_size=P,
                        )
                    Mfb = mpool.tile([P, nbmax * P], BF16, tag="Mfb", name="Mfb3")
                    for b in range(nb):
                        nc.vector.tensor_tensor(
                            out=Mfb[:, b * P:(b + 1) * P],
                            in0=dstloc[:, b0 + b:b0 + b + 1].to_broadcast([P, P]),
                            in1=iotaB[:],
                            op=ALU.is_equal,
                        )
                    m3P = psC.tile([cfg.oc, P], FP32, tag="small", name="m3P")
                    for b in range(nb):
                        nc.tensor.matmul(
                            m3P[:],
                            lhsT=G3[:, b * P: b * P + cfg.oc],
                            rhs=Mfb[:, b * P:(b + 1) * P],
                            start=(b == 0), stop=(b == nb - 1))
                    m3s = spool.tile([cfg.oc, P], BF16, tag="m3s", name="m3s")
                    nc.vector.tensor_tensor(out=m3s[:], in0=m3P[:],
                                            in1=invB[:cfg.oc, :], op=ALU.mult)
                    r3P = psC.tile([cfg.oc, P], FP32, tag="small", name="r3P")
                    for k in range(cfg.h2c):
                        nc.tensor.matmul(
                            r3P[:],
                            lhsT=w3rootb[:, k * cfg.oc:(k + 1) * cfg.oc],
                            rhs=h2T[:, k * cfg.own_pad + t * P: k * cfg.own_pad + (t + 1) * P],
                            start=(k == 0), stop=False)
                    nc.tensor.matmul(r3P[:], lhsT=idb[:cfg.oc, :cfg.oc], rhs=m3s[:],
                                     start=False, stop=True)
                    outS = spool.tile([cfg.oc, P], FP32, tag="outS", name="outS")
                    nc.vector.tensor_scalar_add(out=outS[:], in0=r3P[:],
                                                scalar1=b3T[:cfg.oc, 0:1])
                    nc.sync.dma_start(out=d_out[:, t * P:(t + 1) * P], in_=outS[:])

            except _StageCutExc:
                pass

    nc.compile()
    return nc


_CACHE = {}


def get_compiled(cfg: Cfg, NB):
    key = (cfg.n, cfg.e, cfg.f, cfg.h1, cfg.h2, cfg.out, cfg.nc, tuple(int(x) for x in NB))
    if key not in _CACHE:
        _CACHE[key] = build_kernel(cfg, NB)
    return _CACHE[key]


def unshard(cfg: Cfg, results):
    mu = np.concatenate([r["outT"][:cfg.out, :cfg.own].T for r in results], axis=0)
    ls = np.concatenate([r["outT"][cfg.out:cfg.oc, :cfg.own].T for r in results], axis=0)
    return np.ascontiguousarray(mu), np.ascontiguousarray(ls)


def kernel(**inputs):
    cfg = Cfg(n_nodes=20000, n_edges=160000, f_in=128, h1=1024, h2=512, out=8,
              n_cores=8)
    in_maps, NB = host_prep(cfg, inputs)
    nc = get_compiled(cfg, NB)
    res = run_bass_kernel_spmd(nc, in_maps, core_ids=list(range(cfg.nc)))
    return unshard(cfg, res.results)



# revision 2
# speedup vs baseline: 176.7595x; 176.7595x over previous
"""Trainium2 Bass kernel: 3-layer GraphConv GNN encoder (mean aggregation).

ZERO-COLLECTIVE design: L1 and L2 are fully replicated on every core (each
core computes m1/h1/q2/h2/q3 for ALL nodes); only L3's destination
aggregation is sharded (each core owns a contiguous range of 128-node tiles
and emits mu/logstd for those nodes). No cross-core communication or
synchronization of any kind -> each core's device span is its own work only.

math (PyG GraphConv, aggr='mean'):
    h1 = relu(mean_agg(x) @ w1_rel + b1 + x @ w1_root)
    h2 = relu(mean_agg(q2) + h1 @ w2_root + b2),  q2 = h1 @ w2_rel
    [mu|ls] = mean_agg(q3) + b3 + r3,  q3 = h2 @ w3_rel, r3 = h2 @ w3_root
(mean_agg commutes with the dense projections; aggregate in the smallest
width per layer: x (128), q2 (512), q3 (16).)

Implementation notes:
  - Edges grouped host-side by destination 128-node tile (157 global tiles),
    sorted by source within each tile, padded to 128-edge blocks.
  - mean weights folded into the one-hot scatter matrices: per-edge weight
    1/deg(dst) multiplies the is_equal one-hot (edgew), so aggregation is a
    single accumulated matmul chain per tile — no degree pass on device.
  - Source rows fetched with gpsimd dma_gather from HBM tables
    (x: 256B bf16 rows, q2: 1KB bf16 rows, [q3|r3]: 64B bf16 rows).
  - One-hot builds batched 8 blocks per DVE instruction.
  - The SPMD program is identical on every core; all per-core variation
    (which tiles L3 aggregates, which rows feed the L3 root term) lives in
    per-core index-array INPUTS, so there is no compile-time branching.
"""

import numpy as np
import ml_dtypes

import concourse.bass as bass
import concourse.mybir as mybir
import concourse.tile as tile
from concourse import bacc
from concourse.bass_utils import run_bass_kernel_spmd
from concourse.masks import make_identity

P = 128
FP32 = mybir.dt.float32
BF16 = mybir.dt.bfloat16
I16 = mybir.dt.int16
AF = mybir.ActivationFunctionType
ALU = mybir.AluOpType
BF16NP = ml_dtypes.bfloat16


class Cfg:
    def __init__(self, n_nodes=20000, n_edges=160000, f_in=128, h1=1024, h2=512,
                 out=8, n_cores=8):
        self.n = n_nodes
        self.e = n_edges
        self.f = f_in
        self.h1 = h1
        self.h2 = h2
        self.out = out
        self.nc = n_cores
        self.gsz = 512                             # node-group width for dense
        self.n_pad = ((n_nodes + self.gsz - 1) // self.gsz) * self.gsz
        self.nt = self.n_pad // P                  # global 128-node tiles
        base = self.nt // n_cores
        rem = self.nt % n_cores
        self.tiles_per_core = [base + (1 if c < rem else 0) for c in range(n_cores)]
        self.tile0 = [int(sum(self.tiles_per_core[:c])) for c in range(n_cores)]
        self.max_tiles = max(self.tiles_per_core)
        self.ng = self.n_pad // self.gsz
        self.tpg = self.gsz // P
        self.h1c = h1 // P
        self.h2c = h2 // P
        self.oc = 2 * out                          # mu|logstd concat width (16)


def _wrap_idx(a, dtype=np.int16):
    """dma_gather index layout: idx j at [j%16, j//16], replicated to 128."""
    nb16 = a.shape[0] // 16
    w = a.reshape(nb16, 16).T.astype(dtype)        # [16, nb16]
    return np.tile(w, (8, 1))                      # [128, nb16]


def _group_edges(cfg, src, dst, tiles, tile_base, NB, invdeg):
    """Pack edges (sorted by (dst tile, src)) into padded 128-edge blocks for
    the given list of global tile ids. NB[i] = blocks for tiles[i].
    Returns (srcpad, dstloc, edgew) flat arrays of length sum(NB)*128."""
    nbtot = int(sum(NB))
    srcpad = np.zeros(nbtot * P, dtype=np.int64)
    dstloc = np.full(nbtot * P, -1.0, dtype=np.float32)
    edgew = np.zeros(nbtot * P, dtype=np.float32)
    order = np.lexsort((src, dst))
    src_s, dst_s = src[order], dst[order]
    tile_of = dst_s // P
    off = 0
    for i, t in enumerate(tiles):
        lo = np.searchsorted(tile_of, t)
        hi = np.searchsorted(tile_of, t + 1)
        m = hi - lo
        assert m <= NB[i] * P
        srcpad[off:off + m] = src_s[lo:hi]
        dstloc[off:off + m] = (dst_s[lo:hi] - t * P).astype(np.float32)
        edgew[off:off + m] = invdeg[dst_s[lo:hi]]
        off += NB[i] * P
    return srcpad, dstloc, edgew


def shard_graph(cfg: Cfg, edge_index):
    """Build the global (replicated) edge grouping and the per-core L3
    groupings. Block counts are shared across cores (padded to per-slot max)
    so a single SPMD program fits all cores."""
    src = np.asarray(edge_index[0], dtype=np.int64)
    dst = np.asarray(edge_index[1], dtype=np.int64)
    deg = np.bincount(dst, minlength=cfg.n).astype(np.float64)
    invdeg = (1.0 / np.maximum(deg, 1.0)).astype(np.float32)

    tile_cnt = np.bincount(dst // P, minlength=cfg.nt)
    NB = np.maximum(1, (tile_cnt + P - 1) // P).astype(int)       # len nt

    srcpad, dstloc, edgew = _group_edges(
        cfg, src, dst, list(range(cfg.nt)), 0, NB, invdeg)

    # L3: per-core own tiles; common per-slot block counts
    NB3 = np.zeros(cfg.max_tiles, dtype=int)
    for c in range(cfg.nc):
        t0 = cfg.tile0[c]
        for tl in range(cfg.tiles_per_core[c]):
            NB3[tl] = max(NB3[tl], NB[t0 + tl])
    NB3 = np.maximum(1, NB3)

    per_core = []
    for c in range(cfg.nc):
        t0 = cfg.tile0[c]
        ntl = cfg.tiles_per_core[c]
        tiles = [t0 + tl for tl in range(ntl)]
        # short cores: repeat the last tile to fill max_tiles (output ignored)
        while len(tiles) < cfg.max_tiles:
            tiles.append(tiles[-1])
        s3, d3, w3 = _group_edges(cfg, src, dst, tiles, t0, NB3, invdeg)
        nb3tot = int(NB3.sum())
        # own node ids (for the L3 root-term row gather), one block per tile
        own = np.zeros(cfg.max_tiles * P, dtype=np.int64)
        for i, t in enumerate(tiles):
            own[i * P:(i + 1) * P] = np.arange(t * P, (t + 1) * P)
        own = np.minimum(own, cfg.n_pad - 1)
        per_core.append({
            "idx3": _wrap_idx(s3),                                # [128, nb3tot*8]
            "dstloc3": d3.reshape(nb3tot, P).T.copy(),            # [128, nb3tot]
            "edgew3": w3.reshape(nb3tot, P).T.astype(BF16NP),
            "idxown": _wrap_idx(own),                             # [128, max_tiles*8]
        })

    g = {
        "idx": _wrap_idx(srcpad),
        "dstloc": dstloc.reshape(int(NB.sum()), P).T.copy(),
        "edgew": edgew.reshape(int(NB.sum()), P).T.astype(BF16NP),
    }
    return g, NB, per_core, NB3


def host_prep(cfg: Cfg, inputs):
    x = np.asarray(inputs["x"], dtype=np.float32)
    g, NB, per_core, NB3 = shard_graph(cfg, inputs["edge_index"])

    xgb = np.zeros((cfg.n_pad, cfg.f), dtype=BF16NP)
    xgb[:cfg.n] = x.astype(BF16NP)

    w3rel = np.concatenate([np.asarray(inputs["wmu_rel"]),
                            np.asarray(inputs["wls_rel"])], axis=1)
    w3root = np.concatenate([np.asarray(inputs["wmu_root"]),
                             np.asarray(inputs["wls_root"])], axis=1)
    b3 = np.concatenate([np.asarray(inputs["bmu"]), np.asarray(inputs["bls"])])
    b3T = np.zeros((P, 1), dtype=np.float32)
    b3T[:cfg.oc, 0] = b3
    b1T = np.asarray(inputs["b1"], dtype=np.float32).reshape(cfg.h1c, P).T.copy()
    b2T = np.asarray(inputs["b2"], dtype=np.float32).reshape(cfg.h2c, P).T.copy()

    def chunked(w, kc, wcols):
        # [kc*128, wcols] -> [128, kc*wcols] (k-chunk panels side by side)
        return np.ascontiguousarray(
            np.asarray(w).reshape(kc, P, wcols).transpose(1, 0, 2)
            .reshape(P, kc * wcols)).astype(BF16NP)

    shared = {
        "xgb": xgb,
        "idx": g["idx"],
        "dstloc": g["dstloc"],
        "edgew": g["edgew"],
        "w1rel": np.asarray(inputs["w1_rel"]).astype(BF16NP),
        "w1root": np.asarray(inputs["w1_root"]).astype(BF16NP),
        "w2rel": chunked(inputs["w2_rel"], cfg.h1c, cfg.h2),
        "w2root": chunked(inputs["w2_root"], cfg.h1c, cfg.h2),
        "w3rel": chunked(w3rel, cfg.h2c, cfg.oc),
        "w3root": chunked(w3root, cfg.h2c, cfg.oc),
        "b1T": b1T,
        "b2T": b2T,
        "b3T": b3T,
    }
    in_maps = []
    for c in range(cfg.nc):
        m = dict(shared)
        m.update(per_core[c])
        in_maps.append(m)
    return in_maps, NB, NB3


def build_kernel(cfg: Cfg, NB, NB3):
    NB = [int(v) for v in NB]
    NB3 = [int(v) for v in NB3]
    nbtot = int(sum(NB))
    nbmax = int(max(NB))
    nb3tot = int(sum(NB3))
    base = [int(sum(NB[:t])) for t in range(cfg.nt)]
    base3 = [int(sum(NB3[:t])) for t in range(cfg.max_tiles)]

    nc = bacc.Bacc("TRN2", target_bir_lowering=False, debug=False,
                   num_devices=cfg.nc)

    d_xgb = nc.dram_tensor("xgb", [cfg.n_pad, cfg.f], BF16, kind="ExternalInput")
    d_idx = nc.dram_tensor("idx", [P, nbtot * 8], I16, kind="ExternalInput")
    d_dstloc = nc.dram_tensor("dstloc", [P, nbtot], FP32, kind="ExternalInput")
    d_edgew = nc.dram_tensor("edgew", [P, nbtot], BF16, kind="ExternalInput")
    d_idx3 = nc.dram_tensor("idx3", [P, nb3tot * 8], I16, kind="ExternalInput")
    d_dstloc3 = nc.dram_tensor("dstloc3", [P, nb3tot], FP32, kind="ExternalInput")
    d_edgew3 = nc.dram_tensor("edgew3", [P, nb3tot], BF16, kind="ExternalInput")
    d_idxown = nc.dram_tensor("idxown", [P, cfg.max_tiles * 8], I16,
                              kind="ExternalInput")
    d_w1rel = nc.dram_tensor("w1rel", [cfg.f, cfg.h1], BF16, kind="ExternalInput")
    d_w1root = nc.dram_tensor("w1root", [cfg.f, cfg.h1], BF16, kind="ExternalInput")
    d_w2rel = nc.dram_tensor("w2rel", [P, cfg.h1c * cfg.h2], BF16, kind="ExternalInput")
    d_w2root = nc.dram_tensor("w2root", [P, cfg.h1c * cfg.h2], BF16, kind="ExternalInput")
    d_w3rel = nc.dram_tensor("w3rel", [P, cfg.h2c * cfg.oc], BF16, kind="ExternalInput")
    d_w3root = nc.dram_tensor("w3root", [P, cfg.h2c * cfg.oc], BF16, kind="ExternalInput")
    d_b1T = nc.dram_tensor("b1T", [P, cfg.h1c], FP32, kind="ExternalInput")
    d_b2T = nc.dram_tensor("b2T", [P, cfg.h2c], FP32, kind="ExternalInput")
    d_b3T = nc.dram_tensor("b3T", [P, 1], FP32, kind="ExternalInput")
    d_out = nc.dram_tensor("outT", [cfg.oc, cfg.max_tiles * P], FP32,
                           kind="ExternalOutput")

    with tile.TileContext(nc) as tc:
        with (
            tc.tile_pool(name="const", bufs=1) as cpool,
            tc.tile_pool(name="wts", bufs=1) as wpool,
            tc.tile_pool(name="resid", bufs=1) as rpool,
            tc.tile_pool(name="gat", bufs=3) as gpool,
            tc.tile_pool(name="mwork", bufs=3) as mpool,
            tc.tile_pool(name="stage", bufs=3) as spool,
            tc.tile_pool(name="hstage", bufs=2) as hpool,
            tc.tile_pool(name="psA", bufs=2, space="PSUM") as psA,
            tc.tile_pool(name="psB", bufs=4, space="PSUM") as psB,
            tc.tile_pool(name="psC", bufs=2, space="PSUM") as psC,
            tc.tile_pool(name="dram", bufs=1, space="DRAM") as dpool,
        ):
            # ---- constants ----
            iotaB8 = cpool.tile([P, 8 * P], FP32)
            nc.gpsimd.iota(iotaB8[:].rearrange("p (b j) -> p b j", j=P),
                           pattern=[[0, 8], [1, P]], base=0, channel_multiplier=0,
                           allow_small_or_imprecise_dtypes=True)
            idb = cpool.tile([P, P], BF16)
            make_identity(nc, idb[:])

            b1T = cpool.tile([P, cfg.h1c], FP32)
            nc.sync.dma_start(out=b1T[:], in_=d_b1T[:, :])
            b2T = cpool.tile([P, cfg.h2c], FP32)
            nc.sync.dma_start(out=b2T[:], in_=d_b2T[:, :])
            b3T = cpool.tile([P, 1], FP32)
            nc.sync.dma_start(out=b3T[:], in_=d_b3T[:, :])

            # ---- weights (already bf16) ----
            def wload(dram_ap, cols, tag):
                t = wpool.tile([P, cols], BF16, tag=tag, name=tag)
                nc.sync.dma_start(out=t[:], in_=dram_ap)
                return t

            w1relb = wload(d_w1rel[:, :], cfg.h1, "w1rel")
            w1rootb = wload(d_w1root[:, :], cfg.h1, "w1root")
            w2relb = wload(d_w2rel[:, :], cfg.h1c * cfg.h2, "w2rel")
            w2rootb = wload(d_w2root[:, :], cfg.h1c * cfg.h2, "w2root")
            w3relb = wload(d_w3rel[:, :], cfg.h2c * cfg.oc, "w3rel")
            w3rootb = wload(d_w3root[:, :], cfg.h2c * cfg.oc, "w3root")

            # ---- graph index data ----
            idx = rpool.tile([P, nbtot * 8], I16)
            nc.sync.dma_start(out=idx[:], in_=d_idx[:, :])
            dstloc = rpool.tile([P, nbtot], FP32)
            nc.sync.dma_start(out=dstloc[:], in_=d_dstloc[:, :])
            edgew = rpool.tile([P, nbtot], BF16)
            nc.sync.dma_start(out=edgew[:], in_=d_edgew[:, :])
            idx3 = rpool.tile([P, nb3tot * 8], I16)
            nc.sync.dma_start(out=idx3[:], in_=d_idx3[:, :])
            dstloc3 = rpool.tile([P, nb3tot], FP32)
            nc.sync.dma_start(out=dstloc3[:], in_=d_dstloc3[:, :])
            edgew3 = rpool.tile([P, nb3tot], BF16)
            nc.sync.dma_start(out=edgew3[:], in_=d_edgew3[:, :])
            idxown = rpool.tile([P, cfg.max_tiles * 8], I16)
            nc.sync.dma_start(out=idxown[:], in_=d_idxown[:, :])

            # ---- residents ----
            m1T = rpool.tile([P, cfg.n_pad], BF16)       # mean_agg(x), f-major

            def build_onehot(Mfw, dl, ew, b0, nb):
                for c0 in range(0, nb, 8):
                    cn = min(8, nb - c0)
                    sl = bass.ds(c0 * P, cn * P)
                    nc.vector.tensor_tensor(
                        out=Mfw[:, sl].rearrange("p (b j) -> p b j", j=P),
                        in0=dl[:, b0 + c0:b0 + c0 + cn].to_broadcast([P, cn, P]),
                        in1=iotaB8[:, 0:cn * P].rearrange("p (b j) -> p b j", j=P),
                        op=ALU.is_equal,
                    )
                nc.vector.tensor_tensor(
                    out=Mfw[:, 0:nb * P].rearrange("p (b j) -> p b j", j=P),
                    in0=ew[:, b0:b0 + nb].to_broadcast([P, nb, P]),
                    in1=Mfw[:, 0:nb * P].rearrange("p (b j) -> p b j", j=P),
                    op=ALU.mult,
                )

            def gather_blocks(G, idx_t, b0, nb, table_ap, elem):
                for c0 in range(0, nb, 8):
                    cn = min(8, nb - c0)
                    nc.gpsimd.dma_gather(
                        out_ap=G[:, c0 * elem:(c0 + cn) * elem]
                            .rearrange("p (b e) -> p b e", e=elem),
                        in_ap=table_ap,
                        idxs_ap=idx_t[:, (b0 + c0) * 8:(b0 + c0 + cn) * 8],
                        num_idxs=cn * P,
                        num_idxs_reg=cn * P,
                        elem_size=elem,
                    )

            # ============ Phase A: L1 aggregation, all tiles ============
            for t in range(cfg.nt):
                nb = NB[t]
                b0 = base[t]
                G1 = gpool.tile([P, nbmax * cfg.f], BF16, tag="G", name="G1")
                gather_blocks(G1, idx, b0, nb, d_xgb[:, :], cfg.f)
                Mfw = mpool.tile([P, nbmax * P], BF16, tag="Mf", name="Mfw")
                build_onehot(Mfw, dstloc, edgew, b0, nb)
                m1P = psB.tile([P, P], FP32, tag="t128", name="m1P")
                for b in range(nb):
                    nc.tensor.matmul(
                        m1P[:],
                        lhsT=G1[:, b * cfg.f:(b + 1) * cfg.f],
                        rhs=Mfw[:, b * P:(b + 1) * P],
                        start=(b == 0), stop=(b == nb - 1),
                    )
                nc.scalar.activation(out=m1T[:, t * P:(t + 1) * P], in_=m1P[:],
                                     func=AF.Copy)

            # ============ Phase C: dense L1 -> q2 (node-major) + r2 ============
            q2n = dpool.tile([cfg.n_pad * cfg.h2], BF16, name="q2n")
            q2n2d = q2n[:].rearrange("(r e) -> r e", e=cfg.h2)
            r2f = dpool.tile([cfg.h2 * cfg.n_pad], BF16, name="r2f")
            r2f2d = r2f[:].rearrange("(f n) -> f n", n=cfg.n_pad)

            for g in range(cfg.ng):
                gs = bass.ds(g * cfg.gsz, cfg.gsz)
                # x feature-major for this group (transpose 4 subtiles)
                xg_g = spool.tile([P, cfg.tpg, cfg.f], BF16, tag="xg", name="xg_g")
                nc.sync.dma_start(
                    out=xg_g[:],
                    in_=d_xgb[g * cfg.gsz:(g + 1) * cfg.gsz, :]
                        .rearrange("(s p) e -> p s e", p=P))
                xT_g = spool.tile([P, cfg.gsz], BF16, tag="xT", name="xT_g")
                for s in range(cfg.tpg):
                    tpx = psB.tile([P, P], BF16, tag="t128", name="tpx")
                    nc.tensor.transpose(tpx[:], in_=xg_g[:, s, :], identity=idb[:])
                    nc.vector.tensor_copy(out=xT_g[:, s * P:(s + 1) * P], in_=tpx[:])
                h1s = hpool.tile([P, cfg.h1c * cfg.gsz], BF16, tag="h1s", name="h1s")
                for c in range(cfg.h1c):
                    h1P = psA.tile([P, cfg.gsz], FP32, tag="big", name="h1P")
                    nc.tensor.matmul(h1P[:], lhsT=w1relb[:, c * P:(c + 1) * P],
                                     rhs=m1T[:, gs], start=True, stop=False)
                    nc.tensor.matmul(h1P[:], lhsT=w1rootb[:, c * P:(c + 1) * P],
                                     rhs=xT_g[:], start=False, stop=True)
                    nc.scalar.activation(
                        out=h1s[:, c * cfg.gsz:(c + 1) * cfg.gsz],
                        in_=h1P[:], func=AF.Relu, bias=b1T[:, c:c + 1], scale=1.0)

                q2s = spool.tile([P, cfg.h2c * cfg.gsz], BF16, tag="q2s", name="q2s")
                for c in range(cfg.h2c):
                    q2P = psA.tile([P, cfg.gsz], FP32, tag="big", name="q2P")
                    r2P = psA.tile([P, cfg.gsz], FP32, tag="big", name="r2P")
                    for k in range(cfg.h1c):
                        nc.tensor.matmul(
                            q2P[:],
                            lhsT=w2relb[:, k * cfg.h2 + c * P: k * cfg.h2 + (c + 1) * P],
                            rhs=h1s[:, k * cfg.gsz:(k + 1) * cfg.gsz],
                            start=(k == 0), stop=(k == cfg.h1c - 1))
                    for k in range(cfg.h1c):
                        nc.tensor.matmul(
                            r2P[:],
                            lhsT=w2rootb[:, k * cfg.h2 + c * P: k * cfg.h2 + (c + 1) * P],
                            rhs=h1s[:, k * cfg.gsz:(k + 1) * cfg.gsz],
                            start=(k == 0), stop=(k == cfg.h1c - 1))
                    nc.scalar.activation(out=q2s[:, c * cfg.gsz:(c + 1) * cfg.gsz],
                                         in_=q2P[:], func=AF.Copy)
                    # r2 chunk straight to DRAM (feature-major)
                    r2c = spool.tile([P, cfg.gsz], BF16, tag="r2c", name="r2c")
                    nc.scalar.activation(out=r2c[:], in_=r2P[:], func=AF.Copy)
                    nc.sync.dma_start(
                        out=r2f2d[c * P:(c + 1) * P, g * cfg.gsz:(g + 1) * cfg.gsz],
                        in_=r2c[:])
                # q2 -> node-major rows
                q2nst = spool.tile([P, cfg.tpg, cfg.h2], BF16, tag="q2n", name="q2nst")
                for s in range(cfg.tpg):
                    for c in range(cfg.h2c):
                        tp = psB.tile([P, P], BF16, tag="t128", name="tpq")
                        nc.tensor.transpose(
                            tp[:],
                            in_=q2s[:, c * cfg.gsz + s * P: c * cfg.gsz + (s + 1) * P],
                            identity=idb[:])
                        nc.vector.tensor_copy(out=q2nst[:, s, c * P:(c + 1) * P],
                                              in_=tp[:])
                nc.sync.dma_start(
                    out=q2n2d[g * cfg.gsz:(g + 1) * cfg.gsz, :]
                        .rearrange("(s p) e -> p s e", p=P),
                    in_=q2nst[:])

            # ===== Phase D: L2 scatter + h2 + [q3|r3] rows, all tiles =====
            q3n = dpool.tile([cfg.n_pad * P], BF16, name="q3n")
            q3n2d = q3n[:].rearrange("(r e) -> r e", e=P)

            for t in range(cfg.nt):
                nb = NB[t]
                b0 = base[t]
                G2 = gpool.tile([P, nbmax * cfg.h2], BF16, tag="G", name="G2")
                gather_blocks(G2, idx, b0, nb, q2n2d, cfg.h2)
                Mfw = mpool.tile([P, nbmax * P], BF16, tag="Mf", name="Mfw2")
                build_onehot(Mfw, dstloc, edgew, b0, nb)
                r2sb = spool.tile([P, cfg.h2c * P], BF16, tag="r2sb", name="r2sb")
                nc.sync.dma_start(
                    out=r2sb[:].rearrange("p (c n) -> p c n", n=P),
                    in_=r2f2d[:, t * P:(t + 1) * P]
                        .rearrange("(c p) n -> p c n", p=P))
                h2s = hpool.tile([P, cfg.h2c * P], BF16, tag="h2s", name="h2s")
                for c in range(cfg.h2c):
                    m2P = psB.tile([P, P], FP32, tag="t128", name="m2P")
                    for b in range(nb):
                        nc.tensor.matmul(
                            m2P[:],
                            lhsT=G2[:, b * cfg.h2 + c * P: b * cfg.h2 + (c + 1) * P],
                            rhs=Mfw[:, b * P:(b + 1) * P],
                            start=(b == 0), stop=False)
                    nc.tensor.matmul(m2P[:], lhsT=idb[:],
                                     rhs=r2sb[:, c * P:(c + 1) * P],
                                     start=False, stop=True)
                    nc.scalar.activation(out=h2s[:, c * P:(c + 1) * P],
                                         in_=m2P[:], func=AF.Relu,
                                         bias=b2T[:, c:c + 1], scale=1.0)
                # [q3|r3] tile rows
                q3P = psC.tile([cfg.oc, P], FP32, tag="small", name="q3P")
                r3P = psC.tile([cfg.oc, P], FP32, tag="small", name="r3P")
                for k in range(cfg.h2c):
                    nc.tensor.matmul(
                        q3P[:],
                        lhsT=w3relb[:, k * cfg.oc:(k + 1) * cfg.oc],
                        rhs=h2s[:, k * P:(k + 1) * P],
                        start=(k == 0), stop=(k == cfg.h2c - 1))
                for k in range(cfg.h2c):
                    nc.tensor.matmul(
                        r3P[:],
                        lhsT=w3rootb[:, k * cfg.oc:(k + 1) * cfg.oc],
                        rhs=h2s[:, k * P:(k + 1) * P],
                        start=(k == 0), stop=(k == cfg.h2c - 1))
                qr = spool.tile([cfg.oc, 2 * P], BF16, tag="qr", name="qr")
                nc.vector.tensor_copy(out=qr[:, 0:P], in_=q3P[:])
                nc.vector.tensor_copy(out=qr[:, P:2 * P], in_=r3P[:])
                tp3 = psB.tile([P, 2 * cfg.oc], BF16, tag="t128", name="tp3")
                nc.tensor.transpose(tp3[:, 0:cfg.oc], in_=qr[:, 0:P],
                                    identity=idb[:cfg.oc, :cfg.oc])
                nc.tensor.transpose(tp3[:, cfg.oc:2 * cfg.oc], in_=qr[:, P:2 * P],
                                    identity=idb[:cfg.oc, :cfg.oc])
                q3nst = spool.tile([P, P], BF16, tag="q3st", name="q3nst")
                nc.vector.memset(q3nst[:], 0.0)
                nc.vector.tensor_copy(out=q3nst[:, 0:cfg.oc], in_=tp3[:, 0:cfg.oc])
                nc.vector.tensor_copy(out=q3nst[:, 16:16 + cfg.oc],
                                      in_=tp3[:, cfg.oc:2 * cfg.oc])
                nc.sync.dma_start(out=q3n2d[t * P:(t + 1) * P, :], in_=q3nst[:])

            # ============ Phase E: L3 for own tiles (index-driven) ============
            for tl in range(cfg.max_tiles):
                nb = NB3[tl]
                b0 = base3[tl]
                G3 = gpool.tile([P, nbmax * P], BF16, tag="G3", name="G3")
                gather_blocks(G3, idx3, b0, nb, q3n2d, P)
                Mfw = mpool.tile([P, nbmax * P], BF16, tag="Mf3", name="Mfw3")
                build_onehot(Mfw, dstloc3, edgew3, b0, nb)
                m3P = psC.tile([cfg.oc, P], FP32, tag="small", name="m3P")
                for b in range(nb):
                    nc.tensor.matmul(
                        m3P[:],
                        lhsT=G3[:, b * P: b * P + cfg.oc],
                        rhs=Mfw[:, b * P:(b + 1) * P],
                        start=(b == 0), stop=False)
                # own rows' r3 (node-major) -> transpose -> add
                R3 = gpool.tile([P, P], BF16, tag="G3", name="R3own")
                nc.gpsimd.dma_gather(
                    out_ap=R3[:].rearrange("p (b e) -> p b e", e=P),
                    in_ap=q3n2d,
                    idxs_ap=idxown[:, tl * 8:(tl + 1) * 8],
                    num_idxs=P,
                    num_idxs_reg=P,
                    elem_size=P,
                )
                nc.tensor.matmul(m3P[:], lhsT=R3[:, 16:16 + cfg.oc], rhs=idb[:],
                                 start=False, stop=True)
                outS = spool.tile([cfg.oc, P], FP32, tag="outS", name="outS")
                nc.vector.tensor_scalar_add(out=outS[:], in0=m3P[:],
                                            scalar1=b3T[:cfg.oc, 0:1])
                nc.sync.dma_start(out=d_out[:, tl * P:(tl + 1) * P], in_=outS[:])

    nc.compile()
    return nc


_CACHE = {}


def get_compiled(cfg: Cfg, NB, NB3):
    key = (cfg.n, cfg.e, cfg.f, cfg.h1, cfg.h2, cfg.out, cfg.nc,
           tuple(int(v) for v in NB), tuple(int(v) for v in NB3))
    if key not in _CACHE:
        _CACHE[key] = build_kernel(cfg, NB, NB3)
    return _CACHE[key]


def unshard(cfg: Cfg, results):
    mus, lss = [], []
    for c, r in enumerate(results):
        ncols = cfg.tiles_per_core[c] * P
        lo = cfg.tile0[c] * P
        hi = min(lo + ncols, cfg.n)
        mus.append(r["outT"][:cfg.out, :hi - lo].T)
        lss.append(r["outT"][cfg.out:cfg.oc, :hi - lo].T)
    mu = np.concatenate(mus, axis=0)
    ls = np.concatenate(lss, axis=0)
    return np.ascontiguousarray(mu), np.ascontiguousarray(ls)


def kernel(**inputs):
    cfg = Cfg(n_nodes=20000, n_edges=160000, f_in=128, h1=1024, h2=512, out=8,
              n_cores=8)
    in_maps, NB, NB3 = host_prep(cfg, inputs)
    nc = get_compiled(cfg, NB, NB3)
    res = run_bass_kernel_spmd(nc, in_maps, core_ids=list(range(cfg.nc)))
    return unshard(cfg, res.results)
